# revision 1
# baseline (speedup 1.0000x reference)
"""BiLSTM-CRF loss on 8 Trainium2 NeuronCores.

Strategy:
  - Direction-split: cores 0-3 run the forward LSTM, cores 4-7 the backward
    LSTM (on host-pre-flipped input). Within each group the batch (32) is
    sharded 4 ways -> 8 sequences per core.
  - Device kernel A: input projections x @ W_ih.T + (b_ih+b_hh) as one big
    matmul per core (bias folded in via a ones-row matmul).
  - Device kernel B: 64 unrolled LSTM recurrence steps (compiled once, called
    8x with c/hT state roundtrip). Recurrent matmul is lhsT=h.T (tiny
    stationary), rhs=W_hh.T resident in SBUF; x-projection is folded into the
    same PSUM accumulation group via an identity-stationary matmul.
  - Host (numpy): embedding gather, sequence flips, emissions, CRF
    forward/gold score (cheap, O(T*B*L^2)).
"""
import sys
import numpy as np

sys.path.insert(0, '/opt/trn_rl_repo')

import concourse.bacc as bacc
import concourse.mybir as mybir
from concourse.tile import TileContext
from concourse.bass_utils import run_bass_kernel_spmd
import ml_dtypes

BF16 = ml_dtypes.bfloat16
F32 = np.float32

B, T = 32, 512
V, D, L = 50257, 512, 48
G = 4 * D  # 2048 gate width
NCORES = 8
BL = 8       # sequences per core (dir-split: 4 cores x 8 = 32 per direction)
CH = 128     # recurrence steps per kernel-B invocation
NCH = T // CH
NTOK = T * BL  # tokens per core = 4096
MT = NTOK // 128  # M-tiles in projection = 32

_SIG = mybir.ActivationFunctionType.Sigmoid
_TANH = mybir.ActivationFunctionType.Tanh

_cache = {}


def _build_proj():
    nc = bacc.Bacc()
    dt = mybir.dt
    embT = nc.declare_dram_parameter("embT", [128, 4 * NTOK], dt.bfloat16, isOutput=False)
    wih = nc.declare_dram_parameter("wih", [128, 4 * G], dt.bfloat16, isOutput=False)
    bias = nc.declare_dram_parameter("bias", [1, G], dt.bfloat16, isOutput=False)
    ones = nc.declare_dram_parameter("ones", [1, 128], dt.bfloat16, isOutput=False)
    xp = nc.declare_dram_parameter("xp", [MT, 128, G], dt.bfloat16, isOutput=True)

    with TileContext(nc) as tc:
        with (
            tc.tile_pool(name="const", bufs=1) as cpool,
            tc.tile_pool(name="psum", bufs=2, space="PSUM") as ppool,
            tc.tile_pool(name="out", bufs=3) as opool,
        ):
            embT_sb = cpool.tile([128, 4 * NTOK], dt.bfloat16)
            wih_sb = cpool.tile([128, 4 * G], dt.bfloat16)
            bias_sb = cpool.tile([1, G], dt.bfloat16)
            ones_sb = cpool.tile([1, 128], dt.bfloat16)
            nc.sync.dma_start(out=embT_sb[:], in_=embT[:])
            nc.sync.dma_start(out=wih_sb[:], in_=wih[:])
            nc.sync.dma_start(out=bias_sb[:], in_=bias[:])
            nc.sync.dma_start(out=ones_sb[:], in_=ones[:])
            for m in range(MT):
                ps = ppool.tile([128, G], dt.float32)
                for nb in range(4):
                    o = ps[:, nb * 512:(nb + 1) * 512]
                    for kc in range(4):
                        nc.tensor.matmul(
                            o,
                            embT_sb[:, kc * NTOK + m * 128: kc * NTOK + (m + 1) * 128],
                            wih_sb[:, kc * G + nb * 512: kc * G + (nb + 1) * 512],
                            start=(kc == 0), stop=False)
                    nc.tensor.matmul(
                        o, ones_sb[0:1, :], bias_sb[0:1, nb * 512:(nb + 1) * 512],
                        start=False, stop=True)
                ot = opool.tile([128, G], dt.bfloat16)
                nc.vector.tensor_copy(ot[:], ps[:])
                nc.sync.dma_start(out=xp[m], in_=ot[:])
    nc.finalize()
    return nc


def _build_rec():
    nc = bacc.Bacc()
    dt = mybir.dt
    xpc = nc.declare_dram_parameter("xpc", [CH, 4, BL, 512], dt.bfloat16, isOutput=False)
    whh = nc.declare_dram_parameter("whh", [128, 4 * G], dt.bfloat16, isOutput=False)
    i8 = nc.declare_dram_parameter("i8", [128, 8], dt.bfloat16, isOutput=False)
    c_in = nc.declare_dram_parameter("c_in", [BL, D], dt.float32, isOutput=False)
    hT_in = nc.declare_dram_parameter("hT_in", [128, 4 * BL], dt.bfloat16, isOutput=False)
    hs = nc.declare_dram_parameter("hs", [CH, BL, D], dt.bfloat16, isOutput=True)
    c_out = nc.declare_dram_parameter("c_out", [BL, D], dt.float32, isOutput=True)
    hT_out = nc.declare_dram_parameter("hT_out", [128, 4 * BL], dt.bfloat16, isOutput=True)

    with TileContext(nc) as tc:
        with (
            tc.tile_pool(name="const", bufs=1) as cpool,
            tc.tile_pool(name="xp", bufs=3) as xpool,
            tc.tile_pool(name="state", bufs=2) as spool,
            tc.tile_pool(name="gates", bufs=2) as gpool,
            tc.tile_pool(name="h", bufs=3) as hpool,
            tc.tile_pool(name="pg", bufs=1, space="PSUM") as pgpool,
            tc.tile_pool(name="pt", bufs=2, space="PSUM") as ptpool,
        ):
            whh_sb = cpool.tile([128, 4 * G], dt.bfloat16)
            i8_sb = cpool.tile([128, 8], dt.bfloat16)
            nc.sync.dma_start(out=whh_sb[:], in_=whh[:])
            nc.sync.dma_start(out=i8_sb[:], in_=i8[:])
            c_prev = spool.tile([BL, D], dt.float32, tag="c")
            nc.sync.dma_start(out=c_prev[:], in_=c_in[:])
            hT_prev = spool.tile([128, 4 * BL], dt.bfloat16, tag="hT")
            nc.sync.dma_start(out=hT_prev[:], in_=hT_in[:])

            for j in range(CH):
                xp_sb = xpool.tile([128, 512], dt.bfloat16, tag="xp")
                for nb in range(4):
                    nc.sync.dma_start(out=xp_sb[32 * nb:32 * nb + BL, :],
                                      in_=xpc[j, nb])
                # per-bank PSUM tiles: gate activations start as soon as
                # their own bank's accumulation group finishes
                pgs = [pgpool.tile([BL, 512], dt.float32, tag=f"pg{nb}",
                                   name=f"pg{nb}") for nb in range(4)]
                for nb in range(4):
                    nc.tensor.matmul(
                        pgs[nb][:], i8_sb[32 * nb:32 * nb + BL, :],
                        xp_sb[32 * nb:32 * nb + BL, :], start=True, stop=False,
                        tile_position=(32 * nb, 0))
                acts = []
                for nb in range(4):
                    for kc in range(4):
                        nc.tensor.matmul(
                            pgs[nb][:], hT_prev[:, kc * BL:(kc + 1) * BL],
                            whh_sb[:, kc * G + nb * 512: kc * G + (nb + 1) * 512],
                            start=False, stop=(kc == 3))
                    a_sb = gpool.tile([BL, D], dt.bfloat16, tag=f"act{nb}",
                                      name=f"act{nb}")
                    nc.scalar.activation(a_sb[:], pgs[nb][:],
                                         _TANH if nb == 2 else _SIG)
                    acts.append(a_sb)
                i_sb, f_sb, g_sb, o_sb = acts
                ig = gpool.tile([BL, D], dt.float32, tag="ig")
                nc.vector.tensor_mul(ig[:], i_sb[:], g_sb[:])
                fc = gpool.tile([BL, D], dt.float32, tag="fc")
                nc.vector.tensor_mul(fc[:], f_sb[:], c_prev[:])
                c_new = spool.tile([BL, D], dt.float32, tag="c")
                nc.vector.tensor_add(c_new[:], ig[:], fc[:])
                tc_sb = gpool.tile([BL, D], dt.bfloat16, tag="tc")
                nc.scalar.activation(tc_sb[:], c_new[:], _TANH)
                h_sb = hpool.tile([BL, D], dt.bfloat16, tag="h")
                nc.vector.tensor_mul(h_sb[:], o_sb[:], tc_sb[:])
                nc.sync.dma_start(out=hs[j], in_=h_sb[:])
                pt = ptpool.tile([128, 4 * BL], dt.bfloat16, tag="pt")
                for kc in range(4):
                    nc.tensor.transpose(
                        pt[:, kc * BL:(kc + 1) * BL],
                        h_sb[:, kc * 128:(kc + 1) * 128], i8_sb[0:8, :])
                hT_new = spool.tile([128, 4 * BL], dt.bfloat16, tag="hT")
                nc.vector.tensor_copy(hT_new[:], pt[:])
                c_prev, hT_prev = c_new, hT_new
            nc.sync.dma_start(out=c_out[:], in_=c_prev[:])
            nc.sync.dma_start(out=hT_out[:], in_=hT_prev[:])
    nc.finalize()
    return nc


def _chunk128(a):
    """[512, N] -> [128, 4*N] with k-chunk kc at cols [kc*N:(kc+1)*N]."""
    n = a.shape[1]
    return np.ascontiguousarray(
        a.reshape(4, 128, n).transpose(1, 0, 2).reshape(128, 4 * n))


def _seq_flip(x, lengths):
    t = np.arange(x.shape[1])[None, :]
    idx = lengths[:, None] - 1 - t
    idx = np.where(idx >= 0, idx, t)
    return np.take_along_axis(x, idx[:, :, None], axis=1)


def _logsumexp(a, axis):
    m = np.max(a, axis=axis, keepdims=True)
    return np.squeeze(m, axis) + np.log(np.sum(np.exp(a - m), axis=axis))


def kernel(tokens, tags, lengths, embed, W_ih_f, W_hh_f, b_ih_f, b_hh_f,
           W_ih_b, W_hh_b, b_ih_b, b_hh_b, init_hidden, W_emit, b_emit,
           start_trans, trans, end_trans):
    tokens = np.asarray(tokens).astype(np.int64)
    tags = np.asarray(tags).astype(np.int64)
    lengths = np.asarray(lengths).astype(np.int64)
    embed = np.asarray(embed, F32)

    if "proj" not in _cache:
        _cache["proj"] = _build_proj()
        _cache["rec"] = _build_rec()
    nc_p, nc_r = _cache["proj"], _cache["rec"]

    emb = embed[tokens]                      # [B,T,D] f32
    embr = _seq_flip(emb, lengths)           # reversed input for bwd lstm

    # ---- per-core packing ----
    ones = np.ones((1, 128), BF16)
    i8 = np.zeros((128, 8), BF16)
    for nb in range(4):
        i8[32 * nb:32 * nb + 8] = np.eye(8, dtype=BF16)
    wih_pc, bias_pc, whh_pc, hT0_pc, c0_pc, emb_pc = [], [], [], [], [], []
    for c in range(NCORES):
        d = 0 if c < 4 else 1
        W_ih, W_hh = (W_ih_f, W_hh_f) if d == 0 else (W_ih_b, W_hh_b)
        bsum = (np.asarray(b_ih_f) + np.asarray(b_hh_f)) if d == 0 else \
               (np.asarray(b_ih_b) + np.asarray(b_hh_b))
        wih_pc.append(_chunk128(np.asarray(W_ih, F32).T).astype(BF16))
        whh_pc.append(_chunk128(np.asarray(W_hh, F32).T).astype(BF16))
        bias_pc.append(np.asarray(bsum, F32).reshape(1, G).astype(BF16))
        h0 = np.asarray(init_hidden, F32)[d]          # [D]
        hT0 = np.broadcast_to(h0[:, None], (D, BL))   # [D, BL]
        hT0_pc.append(_chunk128(hT0).astype(BF16))
        c0_pc.append(np.broadcast_to(h0[None, :], (BL, D)).astype(F32).copy())
        x = emb if d == 0 else embr
        sl = x[(c % 4) * BL:(c % 4 + 1) * BL]         # [BL, T, D]
        # [D, T, BL] -> [D, T*BL] (t-major, b-minor) -> chunked
        embT = sl.transpose(2, 1, 0).reshape(D, NTOK)
        emb_pc.append(_chunk128(embT).astype(BF16))

    # ---- projections on device ----
    in_maps = [dict(embT=emb_pc[c], wih=wih_pc[c], bias=bias_pc[c], ones=ones)
               for c in range(NCORES)]
    res = run_bass_kernel_spmd(nc_p, in_maps, core_ids=list(range(NCORES)))
    # xp [MT,128,G] -> [T, BL, G]
    xp_pc = [r["xp"].reshape(T, BL, 4, 512).transpose(0, 2, 1, 3).copy()
             for r in res.results]

    # ---- recurrence: NCH sequential chunk calls ----
    hs_pc = [np.empty((T, BL, D), BF16) for _ in range(NCORES)]
    c_st, hT_st = c0_pc, hT0_pc
    for k in range(NCH):
        in_maps = [dict(xpc=np.ascontiguousarray(xp_pc[c][k * CH:(k + 1) * CH]),
                        whh=whh_pc[c], i8=i8, c_in=c_st[c], hT_in=hT_st[c])
                   for c in range(NCORES)]
        res = run_bass_kernel_spmd(nc_r, in_maps, core_ids=list(range(NCORES)))
        for c in range(NCORES):
            hs_pc[c][k * CH:(k + 1) * CH] = res.results[c]["hs"]
        c_st = [res.results[c]["c_out"] for c in range(NCORES)]
        hT_st = [res.results[c]["hT_out"] for c in range(NCORES)]

    # ---- host epilogue ----
    hf = np.concatenate([hs_pc[c].astype(F32) for c in range(4)], axis=1)   # [T,32,D]
    hbr = np.concatenate([hs_pc[c].astype(F32) for c in range(4, 8)], axis=1)
    hf = hf.transpose(1, 0, 2)            # [B,T,D]
    hb = _seq_flip(hbr.transpose(1, 0, 2), lengths)
    feats = np.concatenate([hf, hb], axis=-1)          # [B,T,2D]
    emissions = feats @ np.asarray(W_emit, F32).T + np.asarray(b_emit, F32)

    e = emissions.astype(np.float64)
    tr = np.asarray(trans, np.float64)
    st = np.asarray(start_trans, np.float64)
    et = np.asarray(end_trans, np.float64)
    mask = np.arange(T)[None, :] < lengths[:, None]
    alpha = e[:, 0] + st
    expTrT = np.exp(tr).T  # [j, i]: new_i = LSE_j(alpha_j + tr[i,j])
    for t in range(1, T):
        m = alpha.max(axis=1, keepdims=True)
        new = e[:, t] + m + np.log(np.exp(alpha - m) @ expTrT)
        alpha = np.where(mask[:, t][:, None], new, alpha)
    fwd = _logsumexp(alpha + et, axis=-1)
    e_tag = np.take_along_axis(e, tags[..., None], axis=-1)[..., 0]
    step_scores = tr[tags[:, 1:], tags[:, :-1]] + e_tag[:, 1:]
    last_tag = np.take_along_axis(tags, (lengths - 1)[:, None], axis=1)[:, 0]
    gold = (st[tags[:, 0]] + e_tag[:, 0]
            + np.sum(np.where(mask[:, 1:], step_scores, 0.0), axis=-1)
            + et[last_tag])
    return np.float32(np.sum(fwd - gold))



# revision 3
# speedup vs baseline: 1.0089x; 1.0089x over previous
"""BiLSTM-CRF loss on 8 TRN2 cores — dual-chain + fp8 DoubleRow recurrence.

Like kernel2 (transposed gates, tanh-half sigmoid trick, fused DVE ops) but
each core runs TWO independent recurrence chains of 4 sequences each,
interleaved to hide the per-step dependency-edge latency (~100ns/edge in the
scheduler's cost model) behind the other chain's engine work.

Per chain X tile [128, 80] f32, block layout [o|i|f|g|C] (16 cols each):
  act1 writes tanh of all four gates into cols 0:64 in ONE instruction;
  P12 = (X[:,16:48]+1) * X[:,48:80] = [(ti+1)*tg | (tf+1)*C]
  C'  = 0.5*P12[16:32] + P12[0:16]  -> next X cols 64:80
  tc  = tanh(0.5*C'); H = (X[:,0:16]+1)*tc
"""
import sys
import numpy as np

sys.path.insert(0, '/opt/trn_rl_repo')

import concourse.bacc as bacc
import concourse.mybir as mybir
from concourse.tile import TileContext
from concourse.bass_utils import run_bass_kernel_spmd
import ml_dtypes

BF16 = ml_dtypes.bfloat16
FP8 = ml_dtypes.float8_e4m3
F32 = np.float32

B, T = 32, 512
V, D, L = 50257, 512, 48
G = 4 * D
NCORES = 8
BL = 8                   # sequences per core
HB = 4                   # sequences per chain
WS = 64                  # steps per window
NW = 8
NCH = T // (WS * NW)

_TANH = mybir.ActivationFunctionType.Tanh
_IDENT = mybir.ActivationFunctionType.Identity
_ADD = mybir.AluOpType.add
_MULT = mybir.AluOpType.mult

_cache = {}

# ps/X block order: o, i, f, g  (gate-chunk mc: i=0..3, f=4..7, g=8..11, o=12..15)
_POS = [1, 2, 3, 0]  # gtype i,f,g,o -> block position


def _ps_region(mc, n):
    gtype, j = divmod(mc, 4)
    base = _POS[gtype] * 4 * n + j * n
    return slice(base, base + n)


def _build_main(nw=NW):
    nc = bacc.Bacc()
    dt = mybir.dt
    wih = nc.declare_dram_parameter("wih", [128, 32, 2, 128], dt.float8e4, isOutput=False)
    whh = nc.declare_dram_parameter("whh", [128, 32, 2, 128], dt.float8e4, isOutput=False)
    bias = nc.declare_dram_parameter("bias", [128, 16], dt.float32, isOutput=False)
    xin = nc.declare_dram_parameter("xin", [nw, 2, 128, 2, WS * BL], dt.float8e4, isOutput=False)
    h0 = nc.declare_dram_parameter("h0", [128, 4, 2, HB], dt.float8e4, isOutput=False)
    c0 = nc.declare_dram_parameter("c0", [128, 4, 2, HB], dt.float32, isOutput=False)
    hs = nc.declare_dram_parameter("hs", [nw, 2, 128, WS, 4, HB], dt.float8e4, isOutput=True)
    c_out = nc.declare_dram_parameter("c_out", [2, 128, 4 * HB], dt.float32, isOutput=True)
    ident = nc.declare_dram_parameter("ident", [128, 128], dt.bfloat16, isOutput=False)

    SC = 4 * HB   # 16 state cols per chain

    with TileContext(nc) as tc:
        with (
            tc.tile_pool(name="const", bufs=1) as cpool,
            tc.tile_pool(name="xw", bufs=2) as xwpool,
            tc.tile_pool(name="xp", bufs=2) as xppool,
            tc.tile_pool(name="hwin", bufs=2) as hwpool,
            tc.tile_pool(name="gact", bufs=2) as gpool,
            tc.tile_pool(name="relay", bufs=2) as rpool,
            tc.tile_pool(name="pp", bufs=2, space="PSUM") as pppool,
            tc.tile_pool(name="pg", bufs=3, space="PSUM") as pgpool,
        ):
            wih_sb = cpool.tile([128, 32, 2, 128], dt.float8e4)
            whh_sb = cpool.tile([128, 32, 2, 128], dt.float8e4)
            bias_sb = cpool.tile([128, 16], dt.float32)
            ident_sb = cpool.tile([128, 128], dt.bfloat16)
            nc.sync.dma_start(out=wih_sb[:], in_=wih[:])
            nc.scalar.dma_start(out=whh_sb[:], in_=whh[:])
            nc.gpsimd.dma_start(out=bias_sb[:], in_=bias[:])
            nc.gpsimd.dma_start(out=ident_sb[:], in_=ident[:])
            h_first = cpool.tile([128, 4, 2, HB], dt.float8e4)
            nc.sync.dma_start(out=h_first[:], in_=h0[:])
            x_cur = []
            for cb in range(2):
                xt = gpool.tile([128, 80], dt.float32, tag=f"x{cb}", name=f"xt{cb}")
                nc.sync.dma_start(out=xt[:, 64:80], in_=c0[:, :, cb, :])
                x_cur.append(xt)

            h_win_prev = [None, None]
            relay_prev = None
            for w in range(nw):
                xw_sb = xwpool.tile([128, 2, 2, WS * BL], dt.float8e4, tag="xw")
                for kcp in range(2):
                    (nc.gpsimd if w == 0 else nc.sync).dma_start(
                        out=xw_sb[:, kcp, :, :], in_=xin[w, kcp])
                xp_win = xppool.tile([128, 16, WS * BL], dt.bfloat16, tag="xp")
                relay_next = rpool.tile([128, 16], dt.float32, tag="rl",
                                        name="relay")
                if w == 0:
                    # piece-major with small copies: nothing else is running,
                    # and the first piece unblocks the recurrence early
                    for pc in range(4):
                        sl = slice(pc * 128, (pc + 1) * 128)
                        for mc in range(16):
                            pp = pppool.tile([128, 128], dt.float32, tag="pp",
                                             name="pp")
                            for kcp in range(2):
                                nc.tensor.matmul(
                                    pp[:], wih_sb[:, kcp * 16 + mc, :, :],
                                    xw_sb[:, kcp, :, sl], start=(kcp == 0),
                                    stop=(kcp == 1),
                                    perf_mode=mybir.MatmulPerfMode.DoubleRow)
                            if mc % 2 == 0:
                                nc.vector.tensor_scalar_add(
                                    xp_win[:, mc, sl], pp[:],
                                    bias_sb[:, mc:mc + 1])
                            else:
                                nc.scalar.activation(
                                    xp_win[:, mc, sl], pp[:], _IDENT,
                                    bias=bias_sb[:, mc:mc + 1])
                else:
                    # full-row psum per gate chunk; two half-row copies per
                    # row, each gated on the previous window's recurrence
                    # progress via the bias relay (spreads the projection
                    # convoy across the window instead of bunching it)
                    for mc in range(16):
                        pp = pppool.tile([128, WS * BL], dt.float32, tag="pp",
                                         name="pp")
                        for pc in range(4):
                            sl = slice(pc * 128, (pc + 1) * 128)
                            for kcp in range(2):
                                nc.tensor.matmul(
                                    pp[:, sl], wih_sb[:, kcp * 16 + mc, :, :],
                                    xw_sb[:, kcp, :, sl], start=(kcp == 0),
                                    stop=(kcp == 1),
                                    perf_mode=mybir.MatmulPerfMode.DoubleRow)
                        hf = WS * BL // 2
                        bcol = relay_prev[:, mc:mc + 1]
                        nc.vector.tensor_scalar_add(
                            xp_win[:, mc, 0:hf], pp[:, 0:hf], bcol)
                        nc.scalar.activation(
                            xp_win[:, mc, hf:], pp[:, hf:], _IDENT, bias=bcol)

                h_win = [hwpool.tile([128, WS, 4, HB], dt.float8e4, tag=f"hw{cb}",
                                     name=f"hw{cb}") for cb in range(2)]
                for tl in range(WS):
                    for cb in range(2):
                        if tl == 0 and w == 0:
                            h_rhs = (lambda cb_: lambda kcp:
                                     h_first[:, 2 * kcp:2 * kcp + 2, cb_, :])(cb)
                        elif tl == 0:
                            h_rhs = (lambda hw_: lambda kcp:
                                     hw_[:, WS - 1, 2 * kcp:2 * kcp + 2, :]
                                     )(h_win_prev[cb])
                        else:
                            h_rhs = (lambda hw_, t_: lambda kcp:
                                     hw_[:, t_ - 1, 2 * kcp:2 * kcp + 2, :]
                                     )(h_win[cb], tl)
                        ps = pgpool.tile([128, 64], dt.float32, tag=f"ps{cb}")
                        # one PSUM accumulation group open at a time: hardware
                        # mis-executes interleaved open groups even though the
                        # scheduler's cost model accepts them
                        for mc in range(16):
                            o = ps[:, _ps_region(mc, HB)]
                            nc.tensor.matmul(
                                o, ident_sb[:],
                                xp_win[:, mc, tl * BL + cb * HB:tl * BL + cb * HB + HB],
                                start=True, stop=False)
                            for kcp in range(2):
                                nc.tensor.matmul(
                                    o, whh_sb[:, kcp * 16 + mc, :, :],
                                    h_rhs(kcp), start=False, stop=(kcp == 1),
                                    perf_mode=mybir.MatmulPerfMode.DoubleRow)
                        xc = x_cur[cb]
                        x_next = gpool.tile([128, 80], dt.float32, tag=f"x{cb}",
                                            name="xt")
                        nc.scalar.activation(xc[:, 0:64], ps[:], _TANH, scale=0.0625)
                        p12 = gpool.tile([128, 2 * SC], dt.float32, tag=f"p12{cb}",
                                         name="p12")
                        nc.vector.scalar_tensor_tensor(
                            p12[:], xc[:, SC:3 * SC], 1.0, xc[:, 3 * SC:5 * SC],
                            _ADD, _MULT)
                        nc.vector.scalar_tensor_tensor(
                            x_next[:, 4 * SC:5 * SC], p12[:, SC:2 * SC], 0.5,
                            p12[:, 0:SC], _MULT, _ADD)
                        tc_sb = gpool.tile([128, SC], dt.bfloat16, tag=f"tc{cb}",
                                           name="tc")
                        nc.scalar.activation(tc_sb[:], x_next[:, 4 * SC:5 * SC],
                                             _TANH, scale=0.5)
                        nc.vector.scalar_tensor_tensor(
                            h_win[cb][:, tl, :, :], xc[:, 0:SC], 1.0,
                            tc_sb[:], _ADD, _MULT)
                        x_cur[cb] = x_next
                        if cb == 0 and w < nw - 1 and tl % 4 == 2:
                            rmc = tl // 4
                            nc.vector.scalar_tensor_tensor(
                                relay_next[:, rmc:rmc + 1],
                                h_win[0][:, tl, 0, 0:1], 0.0,
                                bias_sb[:, rmc:rmc + 1], _MULT, _ADD)
                for cb in range(2):
                    nc.sync.dma_start(out=hs[w, cb], in_=h_win[cb][:])
                    h_win_prev[cb] = h_win[cb]
                relay_prev = relay_next
            for cb in range(2):
                nc.sync.dma_start(out=c_out[cb], in_=x_cur[cb][:, 64:80])
    nc.finalize()
    return nc


def _seq_flip(x, lengths):
    t = np.arange(x.shape[1])[None, :]
    idx = lengths[:, None] - 1 - t
    idx = np.where(idx >= 0, idx, t)
    return np.take_along_axis(x, idx[:, :, None], axis=1)


def _logsumexp(a, axis):
    m = np.max(a, axis=axis, keepdims=True)
    return np.squeeze(m, axis) + np.log(np.sum(np.exp(a - m), axis=axis))


def _pack_lhsT(Wmat):
    Wb = Wmat.reshape(16, 128, 4, 128)          # [mc, m, kc, k]
    return np.ascontiguousarray(
        Wb.transpose(3, 2, 0, 1).reshape(128, 64 * 128))


def kernel(tokens, tags, lengths, embed, W_ih_f, W_hh_f, b_ih_f, b_hh_f,
           W_ih_b, W_hh_b, b_ih_b, b_hh_b, init_hidden, W_emit, b_emit,
           start_trans, trans, end_trans):
    tokens = np.asarray(tokens).astype(np.int64)
    tags = np.asarray(tags).astype(np.int64)
    lengths = np.asarray(lengths).astype(np.int64)
    embed = np.asarray(embed, F32)

    if "main" not in _cache:
        _cache["main"] = _build_main()
    nc_m = _cache["main"]

    emb = embed[tokens]
    embr = _seq_flip(emb, lengths)

    rs = np.ones((G, 1), F32) * 0.5
    rs[2 * D:3 * D] = 1.0
    ident = np.eye(128, dtype=BF16)

    in_maps = []
    for c in range(NCORES):
        d = 0 if c < 4 else 1
        W_ih, W_hh = (W_ih_f, W_hh_f) if d == 0 else (W_ih_b, W_hh_b)
        b_sum = (np.asarray(b_ih_f, F32) + np.asarray(b_hh_f, F32)) if d == 0 else \
                (np.asarray(b_ih_b, F32) + np.asarray(b_hh_b, F32))
        wih_s = np.asarray(W_ih, F32) * rs * 16.0
        whh_s = np.asarray(W_hh, F32) * rs * 0.5 * 16.0
        bias_s = (b_sum * rs[:, 0] * 16.0).reshape(16, 128).T
        h0v = np.asarray(init_hidden, F32)[d]
        H0 = np.broadcast_to(2.0 * h0v.reshape(4, 128).T[:, :, None],
                             (128, 4, BL)).reshape(128, 4 * BL)
        x = emb if d == 0 else embr
        sl = x[(c % 4) * BL:(c % 4 + 1) * BL]
        xin = sl.transpose(2, 1, 0).reshape(2, 2, 128, NW, WS, BL) \
                .transpose(3, 0, 2, 1, 4, 5).reshape(NW, 2, 128, 2, WS * BL)
        def _pack8(W):
            wb = W.reshape(16, 128, 2, 2, 128)
            return np.ascontiguousarray(
                wb.transpose(4, 2, 0, 3, 1).reshape(128, 32, 2, 128))
        in_maps.append(dict(
            wih=_pack8(wih_s).astype(FP8),
            whh=_pack8(whh_s).astype(FP8),
            bias=bias_s.astype(F32),
            xin=np.ascontiguousarray(xin).astype(FP8),
            h0=H0.reshape(128, 4, 2, HB).astype(FP8),
            c0=H0.reshape(128, 4, 2, HB).astype(F32),
            ident=ident))

    res = run_bass_kernel_spmd(nc_m, in_maps, core_ids=list(range(NCORES)))

    hcores = []
    for c in range(NCORES):
        hsv = res.results[c]["hs"].astype(np.float32).reshape(NW, 2, 128, WS, 4, HB)
        h = hsv.transpose(0, 3, 1, 5, 4, 2).reshape(T, BL, D).astype(F32) * 0.5
        hcores.append(h)

    hf = np.concatenate(hcores[:4], axis=1)
    hbr = np.concatenate(hcores[4:], axis=1)
    hf = hf.transpose(1, 0, 2)
    hb = _seq_flip(hbr.transpose(1, 0, 2), lengths)
    feats = np.concatenate([hf, hb], axis=-1)
    emissions = feats @ np.asarray(W_emit, F32).T + np.asarray(b_emit, F32)

    e = emissions.astype(np.float64)
    tr = np.asarray(trans, np.float64)
    st = np.asarray(start_trans, np.float64)
    et = np.asarray(end_trans, np.float64)
    mask = np.arange(T)[None, :] < lengths[:, None]
    alpha = e[:, 0] + st
    expTrT = np.exp(tr).T
    for t in range(1, T):
        m = alpha.max(axis=1, keepdims=True)
        new = e[:, t] + m + np.log(np.exp(alpha - m) @ expTrT)
        alpha = np.where(mask[:, t][:, None], new, alpha)
    fwd = _logsumexp(alpha + et, axis=-1)
    e_tag = np.take_along_axis(e, tags[..., None], axis=-1)[..., 0]
    step_scores = tr[tags[:, 1:], tags[:, :-1]] + e_tag[:, 1:]
    last_tag = np.take_along_axis(tags, (lengths - 1)[:, None], axis=1)[:, 0]
    gold = (st[tags[:, 0]] + e_tag[:, 0]
            + np.sum(np.where(mask[:, 1:], step_scores, 0.0), axis=-1)
            + et[last_tag])
    return np.float32(np.sum(fwd - gold))


# revision 4
# speedup vs baseline: 1.0106x; 1.0018x over previous
"""BiLSTM-CRF loss on 8 TRN2 cores — dual-chain + fp8 DoubleRow recurrence.

Like kernel2 (transposed gates, tanh-half sigmoid trick, fused DVE ops) but
each core runs TWO independent recurrence chains of 4 sequences each,
interleaved to hide the per-step dependency-edge latency (~100ns/edge in the
scheduler's cost model) behind the other chain's engine work.

Per chain X tile [128, 80] f32, block layout [o|i|f|g|C] (16 cols each):
  act1 writes tanh of all four gates into cols 0:64 in ONE instruction;
  P12 = (X[:,16:48]+1) * X[:,48:80] = [(ti+1)*tg | (tf+1)*C]
  C'  = 0.5*P12[16:32] + P12[0:16]  -> next X cols 64:80
  tc  = tanh(0.5*C'); H = (X[:,0:16]+1)*tc
"""
import sys
import numpy as np

sys.path.insert(0, '/opt/trn_rl_repo')

import concourse.bacc as bacc
import concourse.mybir as mybir
from concourse.tile import TileContext
from concourse.bass_utils import run_bass_kernel_spmd
import ml_dtypes

BF16 = ml_dtypes.bfloat16
FP8 = ml_dtypes.float8_e4m3
F32 = np.float32

B, T = 32, 512
V, D, L = 50257, 512, 48
G = 4 * D
NCORES = 8
BL = 8                   # sequences per core
HB = 4                   # sequences per chain
WS = 64                  # steps per window
NW = 8
NCH = T // (WS * NW)

_TANH = mybir.ActivationFunctionType.Tanh
_IDENT = mybir.ActivationFunctionType.Identity
_ADD = mybir.AluOpType.add
_MULT = mybir.AluOpType.mult

_cache = {}

# ps/X block order: o, i, f, g  (gate-chunk mc: i=0..3, f=4..7, g=8..11, o=12..15)
_POS = [1, 2, 3, 0]  # gtype i,f,g,o -> block position


def _ps_region(mc, n):
    gtype, j = divmod(mc, 4)
    base = _POS[gtype] * 4 * n + j * n
    return slice(base, base + n)


def _build_main(nw=NW):
    nc = bacc.Bacc()
    dt = mybir.dt
    wih = nc.declare_dram_parameter("wih", [128, 32, 2, 128], dt.float8e4, isOutput=False)
    whh = nc.declare_dram_parameter("whh", [128, 32, 2, 128], dt.float8e4, isOutput=False)
    bias = nc.declare_dram_parameter("bias", [128, 16], dt.float32, isOutput=False)
    xin = nc.declare_dram_parameter("xin", [nw, 2, 128, 2, WS * BL], dt.float8e4, isOutput=False)
    h0 = nc.declare_dram_parameter("h0", [128, 4, 2, HB], dt.float8e4, isOutput=False)
    c0 = nc.declare_dram_parameter("c0", [128, 4, 2, HB], dt.float32, isOutput=False)
    hs = nc.declare_dram_parameter("hs", [nw, 2, 128, WS, 4, HB], dt.float8e4, isOutput=True)
    c_out = nc.declare_dram_parameter("c_out", [2, 128, 4 * HB], dt.float32, isOutput=True)
    ident = nc.declare_dram_parameter("ident", [128, 128], dt.bfloat16, isOutput=False)

    SC = 4 * HB   # 16 state cols per chain

    with TileContext(nc) as tc:
        with (
            tc.tile_pool(name="const", bufs=1) as cpool,
            tc.tile_pool(name="xw", bufs=2) as xwpool,
            tc.tile_pool(name="xp", bufs=2) as xppool,
            tc.tile_pool(name="hwin", bufs=2) as hwpool,
            tc.tile_pool(name="gact", bufs=2) as gpool,
            tc.tile_pool(name="relay", bufs=2) as rpool,
            tc.tile_pool(name="pp", bufs=2, space="PSUM") as pppool,
            tc.tile_pool(name="pg", bufs=3, space="PSUM") as pgpool,
        ):
            wih_sb = cpool.tile([128, 32, 2, 128], dt.float8e4)
            whh_sb = cpool.tile([128, 32, 2, 128], dt.float8e4)
            bias_sb = cpool.tile([128, 16], dt.float32)
            ident_sb = cpool.tile([128, 128], dt.bfloat16)
            nc.sync.dma_start(out=wih_sb[:, 0:8, :, :], in_=wih[:, 0:8])
            nc.sync.dma_start(out=wih_sb[:, 16:24, :, :], in_=wih[:, 16:24])
            nc.sync.dma_start(out=wih_sb[:, 8:16, :, :], in_=wih[:, 8:16])
            nc.sync.dma_start(out=wih_sb[:, 24:32, :, :], in_=wih[:, 24:32])
            nc.scalar.dma_start(out=whh_sb[:, 0:8, :, :], in_=whh[:, 0:8])
            nc.scalar.dma_start(out=whh_sb[:, 16:24, :, :], in_=whh[:, 16:24])
            nc.scalar.dma_start(out=whh_sb[:, 8:16, :, :], in_=whh[:, 8:16])
            nc.scalar.dma_start(out=whh_sb[:, 24:32, :, :], in_=whh[:, 24:32])
            nc.gpsimd.dma_start(out=bias_sb[:], in_=bias[:])
            nc.gpsimd.dma_start(out=ident_sb[:], in_=ident[:])
            h_first = cpool.tile([128, 4, 2, HB], dt.float8e4)
            nc.sync.dma_start(out=h_first[:], in_=h0[:])
            x_cur = []
            for cb in range(2):
                xt = gpool.tile([128, 80], dt.float32, tag=f"x{cb}", name=f"xt{cb}")
                nc.sync.dma_start(out=xt[:, 64:80], in_=c0[:, :, cb, :])
                x_cur.append(xt)

            h_win_prev = [None, None]
            relay_prev = None
            for w in range(nw):
                xw_sb = xwpool.tile([128, 2, 2, WS * BL], dt.float8e4, tag="xw")
                for kcp in range(2):
                    (nc.gpsimd if w == 0 else nc.sync).dma_start(
                        out=xw_sb[:, kcp, :, :], in_=xin[w, kcp])
                xp_win = xppool.tile([128, 16, WS * BL], dt.bfloat16, tag="xp")
                relay_next = rpool.tile([128, 16], dt.float32, tag="rl",
                                        name="relay")
                if w == 0:
                    # piece-major with small copies: nothing else is running,
                    # and the first piece unblocks the recurrence early
                    for pc in range(4):
                        sl = slice(pc * 128, (pc + 1) * 128)
                        for mc in range(16):
                            pp = pppool.tile([128, 128], dt.float32, tag="pp",
                                             name="pp")
                            for kcp in range(2):
                                nc.tensor.matmul(
                                    pp[:], wih_sb[:, kcp * 16 + mc, :, :],
                                    xw_sb[:, kcp, :, sl], start=(kcp == 0),
                                    stop=(kcp == 1),
                                    perf_mode=mybir.MatmulPerfMode.DoubleRow)
                            if mc % 2 == 0:
                                nc.vector.tensor_scalar_add(
                                    xp_win[:, mc, sl], pp[:],
                                    bias_sb[:, mc:mc + 1])
                            else:
                                nc.scalar.activation(
                                    xp_win[:, mc, sl], pp[:], _IDENT,
                                    bias=bias_sb[:, mc:mc + 1])
                else:
                    # full-row psum per gate chunk; two half-row copies per
                    # row, each gated on the previous window's recurrence
                    # progress via the bias relay (spreads the projection
                    # convoy across the window instead of bunching it)
                    for mc in range(16):
                        pp = pppool.tile([128, WS * BL], dt.float32, tag="pp",
                                         name="pp")
                        for pc in range(4):
                            sl = slice(pc * 128, (pc + 1) * 128)
                            for kcp in range(2):
                                nc.tensor.matmul(
                                    pp[:, sl], wih_sb[:, kcp * 16 + mc, :, :],
                                    xw_sb[:, kcp, :, sl], start=(kcp == 0),
                                    stop=(kcp == 1),
                                    perf_mode=mybir.MatmulPerfMode.DoubleRow)
                        hf = WS * BL // 2
                        bcol = relay_prev[:, mc:mc + 1]
                        nc.vector.tensor_scalar_add(
                            xp_win[:, mc, 0:hf], pp[:, 0:hf], bcol)
                        nc.scalar.activation(
                            xp_win[:, mc, hf:], pp[:, hf:], _IDENT, bias=bcol)

                h_win = [hwpool.tile([128, WS, 4, HB], dt.float8e4, tag=f"hw{cb}",
                                     name=f"hw{cb}") for cb in range(2)]
                for tl in range(WS):
                    for cb in range(2):
                        if tl == 0 and w == 0:
                            h_rhs = (lambda cb_: lambda kcp:
                                     h_first[:, 2 * kcp:2 * kcp + 2, cb_, :])(cb)
                        elif tl == 0:
                            h_rhs = (lambda hw_: lambda kcp:
                                     hw_[:, WS - 1, 2 * kcp:2 * kcp + 2, :]
                                     )(h_win_prev[cb])
                        else:
                            h_rhs = (lambda hw_, t_: lambda kcp:
                                     hw_[:, t_ - 1, 2 * kcp:2 * kcp + 2, :]
                                     )(h_win[cb], tl)
                        ps = pgpool.tile([128, 64], dt.float32, tag=f"ps{cb}")
                        # one PSUM accumulation group open at a time: hardware
                        # mis-executes interleaved open groups even though the
                        # scheduler's cost model accepts them
                        for mc in range(16):
                            o = ps[:, _ps_region(mc, HB)]
                            nc.tensor.matmul(
                                o, ident_sb[:],
                                xp_win[:, mc, tl * BL + cb * HB:tl * BL + cb * HB + HB],
                                start=True, stop=False)
                            for kcp in range(2):
                                nc.tensor.matmul(
                                    o, whh_sb[:, kcp * 16 + mc, :, :],
                                    h_rhs(kcp), start=False, stop=(kcp == 1),
                                    perf_mode=mybir.MatmulPerfMode.DoubleRow)
                        xc = x_cur[cb]
                        x_next = gpool.tile([128, 80], dt.float32, tag=f"x{cb}",
                                            name="xt")
                        nc.scalar.activation(xc[:, 0:64], ps[:], _TANH, scale=0.0625)
                        p12 = gpool.tile([128, 2 * SC], dt.float32, tag=f"p12{cb}",
                                         name="p12")
                        nc.vector.scalar_tensor_tensor(
                            p12[:], xc[:, SC:3 * SC], 1.0, xc[:, 3 * SC:5 * SC],
                            _ADD, _MULT)
                        nc.vector.scalar_tensor_tensor(
                            x_next[:, 4 * SC:5 * SC], p12[:, SC:2 * SC], 0.5,
                            p12[:, 0:SC], _MULT, _ADD)
                        tc_sb = gpool.tile([128, SC], dt.bfloat16, tag=f"tc{cb}",
                                           name="tc")
                        nc.scalar.activation(tc_sb[:], x_next[:, 4 * SC:5 * SC],
                                             _TANH, scale=0.5)
                        nc.vector.scalar_tensor_tensor(
                            h_win[cb][:, tl, :, :], xc[:, 0:SC], 1.0,
                            tc_sb[:], _ADD, _MULT)
                        x_cur[cb] = x_next
                        if cb == 0 and w < nw - 1 and tl % 4 == 2:
                            rmc = tl // 4
                            nc.vector.scalar_tensor_tensor(
                                relay_next[:, rmc:rmc + 1],
                                h_win[0][:, tl, 0, 0:1], 0.0,
                                bias_sb[:, rmc:rmc + 1], _MULT, _ADD)
                for cb in range(2):
                    nc.sync.dma_start(out=hs[w, cb], in_=h_win[cb][:])
                    h_win_prev[cb] = h_win[cb]
                relay_prev = relay_next
            for cb in range(2):
                nc.sync.dma_start(out=c_out[cb], in_=x_cur[cb][:, 64:80])
    nc.finalize()
    return nc


def _seq_flip(x, lengths):
    t = np.arange(x.shape[1])[None, :]
    idx = lengths[:, None] - 1 - t
    idx = np.where(idx >= 0, idx, t)
    return np.take_along_axis(x, idx[:, :, None], axis=1)


def _logsumexp(a, axis):
    m = np.max(a, axis=axis, keepdims=True)
    return np.squeeze(m, axis) + np.log(np.sum(np.exp(a - m), axis=axis))


def _pack_lhsT(Wmat):
    Wb = Wmat.reshape(16, 128, 4, 128)          # [mc, m, kc, k]
    return np.ascontiguousarray(
        Wb.transpose(3, 2, 0, 1).reshape(128, 64 * 128))


def kernel(tokens, tags, lengths, embed, W_ih_f, W_hh_f, b_ih_f, b_hh_f,
           W_ih_b, W_hh_b, b_ih_b, b_hh_b, init_hidden, W_emit, b_emit,
           start_trans, trans, end_trans):
    tokens = np.asarray(tokens).astype(np.int64)
    tags = np.asarray(tags).astype(np.int64)
    lengths = np.asarray(lengths).astype(np.int64)
    embed = np.asarray(embed, F32)

    if "main" not in _cache:
        _cache["main"] = _build_main()
    nc_m = _cache["main"]

    emb = embed[tokens]
    embr = _seq_flip(emb, lengths)

    rs = np.ones((G, 1), F32) * 0.5
    rs[2 * D:3 * D] = 1.0
    ident = np.eye(128, dtype=BF16)

    in_maps = []
    for c in range(NCORES):
        d = 0 if c < 4 else 1
        W_ih, W_hh = (W_ih_f, W_hh_f) if d == 0 else (W_ih_b, W_hh_b)
        b_sum = (np.asarray(b_ih_f, F32) + np.asarray(b_hh_f, F32)) if d == 0 else \
                (np.asarray(b_ih_b, F32) + np.asarray(b_hh_b, F32))
        wih_s = np.asarray(W_ih, F32) * rs * 16.0
        whh_s = np.asarray(W_hh, F32) * rs * 0.5 * 16.0
        bias_s = (b_sum * rs[:, 0] * 16.0).reshape(16, 128).T
        h0v = np.asarray(init_hidden, F32)[d]
        H0 = np.broadcast_to(2.0 * h0v.reshape(4, 128).T[:, :, None],
                             (128, 4, BL)).reshape(128, 4 * BL)
        x = emb if d == 0 else embr
        sl = x[(c % 4) * BL:(c % 4 + 1) * BL]
        xin = sl.transpose(2, 1, 0).reshape(2, 2, 128, NW, WS, BL) \
                .transpose(3, 0, 2, 1, 4, 5).reshape(NW, 2, 128, 2, WS * BL)
        def _pack8(W):
            wb = W.reshape(16, 128, 2, 2, 128)
            return np.ascontiguousarray(
                wb.transpose(4, 2, 0, 3, 1).reshape(128, 32, 2, 128))
        in_maps.append(dict(
            wih=_pack8(wih_s).astype(FP8),
            whh=_pack8(whh_s).astype(FP8),
            bias=bias_s.astype(F32),
            xin=np.ascontiguousarray(xin).astype(FP8),
            h0=H0.reshape(128, 4, 2, HB).astype(FP8),
            c0=H0.reshape(128, 4, 2, HB).astype(F32),
            ident=ident))

    res = run_bass_kernel_spmd(nc_m, in_maps, core_ids=list(range(NCORES)))

    hcores = []
    for c in range(NCORES):
        hsv = res.results[c]["hs"].astype(np.float32).reshape(NW, 2, 128, WS, 4, HB)
        h = hsv.transpose(0, 3, 1, 5, 4, 2).reshape(T, BL, D).astype(F32) * 0.5
        hcores.append(h)

    hf = np.concatenate(hcores[:4], axis=1)
    hbr = np.concatenate(hcores[4:], axis=1)
    hf = hf.transpose(1, 0, 2)
    hb = _seq_flip(hbr.transpose(1, 0, 2), lengths)
    feats = np.concatenate([hf, hb], axis=-1)
    emissions = feats @ np.asarray(W_emit, F32).T + np.asarray(b_emit, F32)

    e = emissions.astype(np.float64)
    tr = np.asarray(trans, np.float64)
    st = np.asarray(start_trans, np.float64)
    et = np.asarray(end_trans, np.float64)
    mask = np.arange(T)[None, :] < lengths[:, None]
    alpha = e[:, 0] + st
    expTrT = np.exp(tr).T
    for t in range(1, T):
        m = alpha.max(axis=1, keepdims=True)
        new = e[:, t] + m + np.log(np.exp(alpha - m) @ expTrT)
        alpha = np.where(mask[:, t][:, None], new, alpha)
    fwd = _logsumexp(alpha + et, axis=-1)
    e_tag = np.take_along_axis(e, tags[..., None], axis=-1)[..., 0]
    step_scores = tr[tags[:, 1:], tags[:, :-1]] + e_tag[:, 1:]
    last_tag = np.take_along_axis(tags, (lengths - 1)[:, None], axis=1)[:, 0]
    gold = (st[tags[:, 0]] + e_tag[:, 0]
            + np.sum(np.where(mask[:, 1:], step_scores, 0.0), axis=-1)
            + et[last_tag])
    return np.float32(np.sum(fwd - gold))


# revision 5
# speedup vs baseline: 1.0239x; 1.0131x over previous
"""BiLSTM-CRF loss on 8 TRN2 cores — dual-chain + fp8 DoubleRow recurrence.

Like kernel2 (transposed gates, tanh-half sigmoid trick, fused DVE ops) but
each core runs TWO independent recurrence chains of 4 sequences each,
interleaved to hide the per-step dependency-edge latency (~100ns/edge in the
scheduler's cost model) behind the other chain's engine work.

Per chain X tile [128, 80] f32, block layout [o|i|f|g|C] (16 cols each):
  act1 writes tanh of all four gates into cols 0:64 in ONE instruction;
  P12 = (X[:,16:48]+1) * X[:,48:80] = [(ti+1)*tg | (tf+1)*C]
  C'  = 0.5*P12[16:32] + P12[0:16]  -> next X cols 64:80
  tc  = tanh(0.5*C'); H = (X[:,0:16]+1)*tc
"""
import sys
import numpy as np

sys.path.insert(0, '/opt/trn_rl_repo')

import concourse.bacc as bacc
import concourse.mybir as mybir
from concourse.tile import TileContext
from concourse.bass_utils import run_bass_kernel_spmd
import ml_dtypes

BF16 = ml_dtypes.bfloat16
FP8 = ml_dtypes.float8_e4m3
F32 = np.float32

B, T = 32, 512
V, D, L = 50257, 512, 48
G = 4 * D
NCORES = 8
BL = 8                   # sequences per core
HB = 4                   # sequences per chain
WS = 64                  # steps per window
NW = 8
NCH = T // (WS * NW)

_TANH = mybir.ActivationFunctionType.Tanh
_IDENT = mybir.ActivationFunctionType.Identity
_ADD = mybir.AluOpType.add
_MULT = mybir.AluOpType.mult

_cache = {}

# ps/X block order: o, i, f, g  (gate-chunk mc: i=0..3, f=4..7, g=8..11, o=12..15)
_POS = [1, 2, 3, 0]  # gtype i,f,g,o -> block position


def _ps_region(mc, n):
    gtype, j = divmod(mc, 4)
    base = _POS[gtype] * 4 * n + j * n
    return slice(base, base + n)


def _build_main(nw=NW, last_ws=WS):
    nc = bacc.Bacc()
    dt = mybir.dt
    wih = nc.declare_dram_parameter("wih", [128, 32, 2, 128], dt.float8e4, isOutput=False)
    whh = nc.declare_dram_parameter("whh", [128, 32, 2, 128], dt.float8e4, isOutput=False)
    bias = nc.declare_dram_parameter("bias", [128, 16], dt.float32, isOutput=False)
    xin = nc.declare_dram_parameter("xin", [nw, 2, 128, 2, WS * BL], dt.float8e4, isOutput=False)
    h0 = nc.declare_dram_parameter("h0", [128, 4, 2, HB], dt.float8e4, isOutput=False)
    c0 = nc.declare_dram_parameter("c0", [128, 4, 2, HB], dt.float32, isOutput=False)
    hs = nc.declare_dram_parameter("hs", [nw, 2, 128, WS, 4, HB], dt.float8e4, isOutput=True)
    c_out = nc.declare_dram_parameter("c_out", [2, 128, 4 * HB], dt.float32, isOutput=True)
    ident = nc.declare_dram_parameter("ident", [128, 128], dt.bfloat16, isOutput=False)

    SC = 4 * HB   # 16 state cols per chain

    with TileContext(nc) as tc:
        with (
            tc.tile_pool(name="const", bufs=1) as cpool,
            tc.tile_pool(name="xw", bufs=2) as xwpool,
            tc.tile_pool(name="xp", bufs=2) as xppool,
            tc.tile_pool(name="hwin", bufs=2) as hwpool,
            tc.tile_pool(name="gact", bufs=2) as gpool,
            tc.tile_pool(name="relay", bufs=2) as rpool,
            tc.tile_pool(name="pp", bufs=2, space="PSUM") as pppool,
            tc.tile_pool(name="pg", bufs=3, space="PSUM") as pgpool,
        ):
            wih_sb = cpool.tile([128, 32, 2, 128], dt.float8e4)
            whh_sb = cpool.tile([128, 32, 2, 128], dt.float8e4)
            bias_sb = cpool.tile([128, 16], dt.float32)
            ident_sb = cpool.tile([128, 128], dt.bfloat16)
            nc.sync.dma_start(out=wih_sb[:, 0:8, :, :], in_=wih[:, 0:8])
            nc.sync.dma_start(out=wih_sb[:, 16:24, :, :], in_=wih[:, 16:24])
            nc.sync.dma_start(out=wih_sb[:, 8:16, :, :], in_=wih[:, 8:16])
            nc.sync.dma_start(out=wih_sb[:, 24:32, :, :], in_=wih[:, 24:32])
            nc.scalar.dma_start(out=whh_sb[:, 0:8, :, :], in_=whh[:, 0:8])
            nc.scalar.dma_start(out=whh_sb[:, 16:24, :, :], in_=whh[:, 16:24])
            nc.scalar.dma_start(out=whh_sb[:, 8:16, :, :], in_=whh[:, 8:16])
            nc.scalar.dma_start(out=whh_sb[:, 24:32, :, :], in_=whh[:, 24:32])
            nc.gpsimd.dma_start(out=bias_sb[:], in_=bias[:])
            nc.gpsimd.dma_start(out=ident_sb[:], in_=ident[:])
            h_first = cpool.tile([128, 4, 2, HB], dt.float8e4)
            nc.sync.dma_start(out=h_first[:], in_=h0[:])
            x_cur = []
            for cb in range(2):
                xt = gpool.tile([128, 80], dt.float32, tag=f"x{cb}", name=f"xt{cb}")
                nc.sync.dma_start(out=xt[:, 64:80], in_=c0[:, :, cb, :])
                x_cur.append(xt)

            h_win_prev = [None, None]
            relay_prev = None
            for w in range(nw):
                xw_sb = xwpool.tile([128, 2, 2, WS * BL], dt.float8e4, tag="xw")
                for kcp in range(2):
                    (nc.gpsimd if w == 0 else nc.sync).dma_start(
                        out=xw_sb[:, kcp, :, :], in_=xin[w, kcp])
                xp_win = xppool.tile([128, 16, WS * BL], dt.bfloat16, tag="xp")
                relay_next = rpool.tile([128, 16], dt.float32, tag="rl",
                                        name="relay")
                if w == 0:
                    # piece-major with small copies: nothing else is running,
                    # and the first piece unblocks the recurrence early
                    for pc in range(4):
                        sl = slice(pc * 128, (pc + 1) * 128)
                        for mc in range(16):
                            pp = pppool.tile([128, 128], dt.float32, tag="pp",
                                             name="pp")
                            for kcp in range(2):
                                nc.tensor.matmul(
                                    pp[:], wih_sb[:, kcp * 16 + mc, :, :],
                                    xw_sb[:, kcp, :, sl], start=(kcp == 0),
                                    stop=(kcp == 1),
                                    perf_mode=mybir.MatmulPerfMode.DoubleRow)
                            if mc % 2 == 0:
                                nc.vector.tensor_scalar_add(
                                    xp_win[:, mc, sl], pp[:],
                                    bias_sb[:, mc:mc + 1])
                            else:
                                nc.scalar.activation(
                                    xp_win[:, mc, sl], pp[:], _IDENT,
                                    bias=bias_sb[:, mc:mc + 1])
                else:
                    # full-row psum per gate chunk; two half-row copies per
                    # row, each gated on the previous window's recurrence
                    # progress via the bias relay (spreads the projection
                    # convoy across the window instead of bunching it)
                    for mc in range(16):
                        pp = pppool.tile([128, WS * BL], dt.float32, tag="pp",
                                         name="pp")
                        for pc in range(4):
                            sl = slice(pc * 128, (pc + 1) * 128)
                            for kcp in range(2):
                                nc.tensor.matmul(
                                    pp[:, sl], wih_sb[:, kcp * 16 + mc, :, :],
                                    xw_sb[:, kcp, :, sl], start=(kcp == 0),
                                    stop=(kcp == 1),
                                    perf_mode=mybir.MatmulPerfMode.DoubleRow)
                        hf = WS * BL // 2
                        bcol = relay_prev[:, mc:mc + 1]
                        nc.vector.tensor_scalar_add(
                            xp_win[:, mc, 0:hf], pp[:, 0:hf], bcol)
                        nc.scalar.activation(
                            xp_win[:, mc, hf:], pp[:, hf:], _IDENT, bias=bcol)

                h_win = [hwpool.tile([128, WS, 4, HB], dt.float8e4, tag=f"hw{cb}",
                                     name=f"hw{cb}") for cb in range(2)]
                n_tl = last_ws if w == nw - 1 else WS
                for tl in range(n_tl):
                    for cb in range(2):
                        if tl == 0 and w == 0:
                            h_rhs = (lambda cb_: lambda kcp:
                                     h_first[:, 2 * kcp:2 * kcp + 2, cb_, :])(cb)
                        elif tl == 0:
                            h_rhs = (lambda hw_: lambda kcp:
                                     hw_[:, WS - 1, 2 * kcp:2 * kcp + 2, :]
                                     )(h_win_prev[cb])
                        else:
                            h_rhs = (lambda hw_, t_: lambda kcp:
                                     hw_[:, t_ - 1, 2 * kcp:2 * kcp + 2, :]
                                     )(h_win[cb], tl)
                        ps = pgpool.tile([128, 64], dt.float32, tag=f"ps{cb}")
                        # one PSUM accumulation group open at a time: hardware
                        # mis-executes interleaved open groups even though the
                        # scheduler's cost model accepts them
                        for mc in range(16):
                            o = ps[:, _ps_region(mc, HB)]
                            nc.tensor.matmul(
                                o, ident_sb[:],
                                xp_win[:, mc, tl * BL + cb * HB:tl * BL + cb * HB + HB],
                                start=True, stop=False)
                            for kcp in range(2):
                                nc.tensor.matmul(
                                    o, whh_sb[:, kcp * 16 + mc, :, :],
                                    h_rhs(kcp), start=False, stop=(kcp == 1),
                                    perf_mode=mybir.MatmulPerfMode.DoubleRow)
                        xc = x_cur[cb]
                        x_next = gpool.tile([128, 80], dt.float32, tag=f"x{cb}",
                                            name="xt")
                        nc.scalar.activation(xc[:, 0:64], ps[:], _TANH, scale=0.0625)
                        p12 = gpool.tile([128, 2 * SC], dt.float32, tag=f"p12{cb}",
                                         name="p12")
                        nc.vector.scalar_tensor_tensor(
                            p12[:], xc[:, SC:3 * SC], 1.0, xc[:, 3 * SC:5 * SC],
                            _ADD, _MULT)
                        nc.vector.scalar_tensor_tensor(
                            x_next[:, 4 * SC:5 * SC], p12[:, SC:2 * SC], 0.5,
                            p12[:, 0:SC], _MULT, _ADD)
                        tc_sb = gpool.tile([128, SC], dt.bfloat16, tag=f"tc{cb}",
                                           name="tc")
                        nc.scalar.activation(tc_sb[:], x_next[:, 4 * SC:5 * SC],
                                             _TANH, scale=0.5)
                        nc.vector.scalar_tensor_tensor(
                            h_win[cb][:, tl, :, :], xc[:, 0:SC], 1.0,
                            tc_sb[:], _ADD, _MULT)
                        x_cur[cb] = x_next
                        if cb == 0 and w < nw - 1 and tl % 4 == 2:
                            rmc = tl // 4
                            nc.vector.scalar_tensor_tensor(
                                relay_next[:, rmc:rmc + 1],
                                h_win[0][:, tl, 0, 0:1], 0.0,
                                bias_sb[:, rmc:rmc + 1], _MULT, _ADD)
                for cb in range(2):
                    nc.sync.dma_start(out=hs[w, cb], in_=h_win[cb][:])
                    h_win_prev[cb] = h_win[cb]
                relay_prev = relay_next
            for cb in range(2):
                nc.sync.dma_start(out=c_out[cb], in_=x_cur[cb][:, 64:80])
    nc.finalize()
    return nc


def _seq_flip(x, lengths):
    t = np.arange(x.shape[1])[None, :]
    idx = lengths[:, None] - 1 - t
    idx = np.where(idx >= 0, idx, t)
    return np.take_along_axis(x, idx[:, :, None], axis=1)


def _logsumexp(a, axis):
    m = np.max(a, axis=axis, keepdims=True)
    return np.squeeze(m, axis) + np.log(np.sum(np.exp(a - m), axis=axis))


def _pack_lhsT(Wmat):
    Wb = Wmat.reshape(16, 128, 4, 128)          # [mc, m, kc, k]
    return np.ascontiguousarray(
        Wb.transpose(3, 2, 0, 1).reshape(128, 64 * 128))


def kernel(tokens, tags, lengths, embed, W_ih_f, W_hh_f, b_ih_f, b_hh_f,
           W_ih_b, W_hh_b, b_ih_b, b_hh_b, init_hidden, W_emit, b_emit,
           start_trans, trans, end_trans):
    tokens = np.asarray(tokens).astype(np.int64)
    tags = np.asarray(tags).astype(np.int64)
    lengths = np.asarray(lengths).astype(np.int64)
    embed = np.asarray(embed, F32)

    m_len = int(lengths.max())
    lws = min(WS, max(1, m_len - (NW - 1) * WS))
    key = f"main{lws}"
    if key not in _cache:
        _cache[key] = _build_main(last_ws=lws)
    nc_m = _cache[key]

    emb = embed[tokens]
    embr = _seq_flip(emb, lengths)

    rs = np.ones((G, 1), F32) * 0.5
    rs[2 * D:3 * D] = 1.0
    ident = np.eye(128, dtype=BF16)

    in_maps = []
    for c in range(NCORES):
        d = 0 if c < 4 else 1
        W_ih, W_hh = (W_ih_f, W_hh_f) if d == 0 else (W_ih_b, W_hh_b)
        b_sum = (np.asarray(b_ih_f, F32) + np.asarray(b_hh_f, F32)) if d == 0 else \
                (np.asarray(b_ih_b, F32) + np.asarray(b_hh_b, F32))
        wih_s = np.asarray(W_ih, F32) * rs * 16.0
        whh_s = np.asarray(W_hh, F32) * rs * 0.5 * 16.0
        bias_s = (b_sum * rs[:, 0] * 16.0).reshape(16, 128).T
        h0v = np.asarray(init_hidden, F32)[d]
        H0 = np.broadcast_to(2.0 * h0v.reshape(4, 128).T[:, :, None],
                             (128, 4, BL)).reshape(128, 4 * BL)
        x = emb if d == 0 else embr
        sl = x[(c % 4) * BL:(c % 4 + 1) * BL]
        xin = sl.transpose(2, 1, 0).reshape(2, 2, 128, NW, WS, BL) \
                .transpose(3, 0, 2, 1, 4, 5).reshape(NW, 2, 128, 2, WS * BL)
        def _pack8(W):
            wb = W.reshape(16, 128, 2, 2, 128)
            return np.ascontiguousarray(
                wb.transpose(4, 2, 0, 3, 1).reshape(128, 32, 2, 128))
        in_maps.append(dict(
            wih=_pack8(wih_s).astype(FP8),
            whh=_pack8(whh_s).astype(FP8),
            bias=bias_s.astype(F32),
            xin=np.ascontiguousarray(xin).astype(FP8),
            h0=H0.reshape(128, 4, 2, HB).astype(FP8),
            c0=H0.reshape(128, 4, 2, HB).astype(F32),
            ident=ident))

    res = run_bass_kernel_spmd(nc_m, in_maps, core_ids=list(range(NCORES)))

    hcores = []
    for c in range(NCORES):
        hsv = res.results[c]["hs"].astype(np.float32).reshape(NW, 2, 128, WS, 4, HB)
        h = hsv.transpose(0, 3, 1, 5, 4, 2).reshape(T, BL, D).astype(F32) * 0.5
        hcores.append(h)

    hf = np.concatenate(hcores[:4], axis=1)
    hbr = np.concatenate(hcores[4:], axis=1)
    hf = hf.transpose(1, 0, 2)
    hb = _seq_flip(hbr.transpose(1, 0, 2), lengths)
    feats = np.concatenate([hf, hb], axis=-1)
    emissions = feats @ np.asarray(W_emit, F32).T + np.asarray(b_emit, F32)

    e = emissions.astype(np.float64)
    tr = np.asarray(trans, np.float64)
    st = np.asarray(start_trans, np.float64)
    et = np.asarray(end_trans, np.float64)
    mask = np.arange(T)[None, :] < lengths[:, None]
    alpha = e[:, 0] + st
    expTrT = np.exp(tr).T
    for t in range(1, T):
        m = alpha.max(axis=1, keepdims=True)
        new = e[:, t] + m + np.log(np.exp(alpha - m) @ expTrT)
        alpha = np.where(mask[:, t][:, None], new, alpha)
    fwd = _logsumexp(alpha + et, axis=-1)
    e_tag = np.take_along_axis(e, tags[..., None], axis=-1)[..., 0]
    step_scores = tr[tags[:, 1:], tags[:, :-1]] + e_tag[:, 1:]
    last_tag = np.take_along_axis(tags, (lengths - 1)[:, None], axis=1)[:, 0]
    gold = (st[tags[:, 0]] + e_tag[:, 0]
            + np.sum(np.where(mask[:, 1:], step_scores, 0.0), axis=-1)
            + et[last_tag])
    return np.float32(np.sum(fwd - gold))
